# revision 1
# baseline (speedup 1.0000x reference)
"""Trainium2 Bass kernel for nn_DGCN (gnn_message_passing).

Reference computation (C=128, N=1024, T=256, D=2):
    xc  = conv_w @ x + conv_b                    (pointwise channel mix)
    adj = graph_generator(xc, memory, fc_w, fc_b)  ([N,N], top-819 row mask)
    cur1 = xc @ adj;  cur2 = cur1 @ adj          (diffusion over nodes)
    xg  = gcn_w @ [cur1; cur2] + gcn_b
    out = xg * emb + x

Distribution: time axis T sharded 8 ways (32 t per core). The [N,N]
adjacency needs xs = sum_t xc = conv_w @ (sum_t x) + T*conv_b: each core
computes a partial sum over its T-slice, AllReduce(xs), then each core
computes its own 128 rows of the adjacency (softmaxes + exact top-k mask)
and an AllGather replicates the full adjacency. Diffusion/gcn/output run
on the local T-slice.

Top-k (k=819) exact reproduction: the threshold value is always the
row-minimum of af (the ~512-entry group where both relus are zero gives
identical floats); jax.lax.top_k keeps lowest-index ties first. So:
mask = (af > min) | (is_min & (prefix_count(is_min) <= count_min - 205)).
Verified bit-exact vs jax on the reference inputs.

All heavy matmuls run in float32r (TF32-like, 11-bit mantissa, full PE
rate at moving dim >= 256); the small sign-critical matmuls (xs, e1, e2)
run in fp32. Output error lands ~1e-4 relative, inside the fp32 envelope.
"""
import numpy as np

import concourse.bacc as bacc
import concourse.bass as bass
import concourse.mybir as mybir
import concourse.tile as tile
from concourse import bass_utils

f32 = mybir.dt.float32
f32r = mybir.dt.float32r
AX = mybir.AxisListType
OP = mybir.AluOpType
AF = mybir.ActivationFunctionType

C, N, T, D = 128, 1024, 256, 2
NCORES = 8
TS = T // NCORES          # 32 t per core
TB = 4                    # t per block
NBLK = TS // TB           # 8 blocks
K = int(N * 0.8)          # 819
NK = N - K                # 205
NT = N // 128             # 8 n-tiles
SCALE = float(1.0 / np.sqrt(N))


def build_kernel(R=1, sim_mode=False, parts=("p1", "adj", "p2")):
    nc = bacc.Bacc("TRN2", target_bir_lowering=False, debug=False,
                   num_devices=NCORES)
    # --- DRAM I/O (per core) ---
    xin = nc.dram_tensor("xin", [C, TS, N], f32, kind="ExternalInput").ap()
    embi = nc.dram_tensor("embi", [C, TS, N], f32, kind="ExternalInput").ap()
    memi = nc.dram_tensor("memi", [C, N], f32, kind="ExternalInput").ap()
    cwTi = nc.dram_tensor("cwTi", [C, C], f32, kind="ExternalInput").ap()
    gw1Ti = nc.dram_tensor("gw1Ti", [C, C], f32, kind="ExternalInput").ap()
    gw2Ti = nc.dram_tensor("gw2Ti", [C, C], f32, kind="ExternalInput").ap()
    identi = nc.dram_tensor("identi", [C, C], f32, kind="ExternalInput").ap()
    cbi = nc.dram_tensor("cbi", [C, 1], f32, kind="ExternalInput").ap()
    Tcbi = nc.dram_tensor("Tcbi", [C, 1], f32, kind="ExternalInput").ap()
    gbi = nc.dram_tensor("gbi", [C, 1], f32, kind="ExternalInput").ap()
    w0bi = nc.dram_tensor("w0bi", [C, 1], f32, kind="ExternalInput").ap()
    w1bi = nc.dram_tensor("w1bi", [C, 1], f32, kind="ExternalInput").ap()
    fcbbi = nc.dram_tensor("fcbbi", [C, 1], f32, kind="ExternalInput").ap()
    outp = nc.dram_tensor("outp", [C, TS, N], f32, kind="ExternalOutput").ap()

    with tile.TileContext(nc) as tc:
        with (
            tc.tile_pool(name="constp", bufs=1) as constp,
            tc.tile_pool(name="colp", bufs=16) as colp,
            tc.tile_pool(name="scratch", bufs=5) as scratch,
            tc.tile_pool(name="stream", bufs=3) as stream,
            tc.tile_pool(name="embp", bufs=3) as embp,
            tc.tile_pool(name="rnat", bufs=4) as rnat,
            tc.tile_pool(name="rnat2", bufs=4) as rnat2,
            tc.tile_pool(name="xcTp", bufs=2) as xcTp,
            tc.tile_pool(name="c1Tp", bufs=2) as c1Tp,
            tc.tile_pool(name="ps", bufs=2, space="PSUM") as ps,
            tc.tile_pool(name="dram", bufs=1, space="DRAM") as dram,
        ):
            # --- constants ---
            cwTr = constp.tile([C, C], f32r, tag="cwTr")
            nc.gpsimd.dma_start(cwTr[:], cwTi)
            cwT32 = constp.tile([C, C], f32, tag="cwT32")
            nc.sync.dma_start(cwT32[:], cwTi)
            gw1Tr = constp.tile([C, C], f32r, tag="gw1Tr")
            nc.gpsimd.dma_start(gw1Tr[:], gw1Ti)
            gw2Tr = constp.tile([C, C], f32r, tag="gw2Tr")
            nc.gpsimd.dma_start(gw2Tr[:], gw2Ti)
            identr = constp.tile([C, C], f32r, tag="identr")
            nc.gpsimd.dma_start(identr[:], identi)
            mem32 = constp.tile([C, N], f32, tag="mem32")
            nc.sync.dma_start(mem32[:], memi)
            cb = constp.tile([C, 1], f32, tag="cb")
            nc.sync.dma_start(cb[:], cbi)
            Tcb = constp.tile([C, 1], f32, tag="Tcb")
            nc.sync.dma_start(Tcb[:], Tcbi)
            gb = constp.tile([C, 1], f32, tag="gb")
            nc.sync.dma_start(gb[:], gbi)
            w0b = constp.tile([C, 1], f32, tag="w0b")
            nc.sync.dma_start(w0b[:], w0bi)
            w1b = constp.tile([C, 1], f32, tag="w1b")
            nc.sync.dma_start(w1b[:], w1bi)
            fcbb = constp.tile([C, 1], f32, tag="fcbb")
            nc.sync.dma_start(fcbb[:], fcbbi)
            adj_all = constp.tile([C, NT * N], f32r, tag="adj_all")  # 32KB
            sxp = constp.tile([C, N], f32, tag="sxp")
            xs_full = constp.tile([C, N], f32, tag="xs_full")
            xs_own = constp.tile([C, C], f32, tag="xs_own")

            # DRAM scratch
            spill = dram.tile([N, NBLK * 512], f32r, tag="spill")  # xcT 16MB
            xs_in = dram.tile([C, N], f32, tag="xs_in")
            xs_out = dram.tile([C, N], f32, tag="xs_out")
            ag_in = dram.tile([C, N], f32r, tag="ag_in")
            ag_out = dram.tile([N, N], f32r, tag="ag_out")

            def pass1():
                nc.vector.memset(sxp[:], 0.0)
                for b in range(NBLK):
                    xcT = xcTp.tile([C, NT * 512], f32r, tag="xcT",
                                    name=f"xcT_p1_{b}")
                    for tl in range(TB):
                        t = b * TB + tl
                        xt = stream.tile([C, N], f32, tag="xt",
                                         name=f"xt_{t}")
                        nc.sync.dma_start(xt[:], xin[:, t, :])
                        nc.vector.tensor_tensor(sxp[:], sxp[:], xt[:], OP.add)
                        xr = rnat.tile([C, N], f32r, tag="xr",
                                       name=f"xr_{t}")
                        nc.scalar.copy(xr[:], xt[:])
                        xc = rnat2.tile([C, N], f32r, tag="xc",
                                        name=f"xc_{t}")
                        for h in range(2):
                            pc = ps.tile([128, 512], f32, tag="mmA",
                                         name=f"pc_{t}_{h}")
                            nc.tensor.matmul(pc[:], cwTr[:],
                                             xr[:, h * 512:(h + 1) * 512],
                                             start=True, stop=True)
                            nc.vector.tensor_scalar_add(
                                xc[:, h * 512:(h + 1) * 512], pc[:], cb[:])
                        for g in range(2):
                            pt = ps.tile([128, 512], f32r, tag="tr",
                                         name=f"pt_{t}_{g}")
                            for jj in range(4):
                                j = g * 4 + jj
                                nc.tensor.transpose(
                                    pt[:, jj * 128:(jj + 1) * 128],
                                    xc[:, j * 128:(j + 1) * 128], identr[:])
                            for jj in range(4):
                                j = g * 4 + jj
                                nc.vector.tensor_copy(
                                    xcT[:, j * 512 + tl * 128:
                                        j * 512 + (tl + 1) * 128],
                                    pt[:, jj * 128:(jj + 1) * 128])
                    for j in range(NT):
                        nc.sync.dma_start(
                            spill[j * 128:(j + 1) * 128,
                                  b * 512:(b + 1) * 512],
                            xcT[:, j * 512:(j + 1) * 512])

            def adjacency():
                # xs = conv_w @ sx + T*conv_b, then AllReduce
                xs_sb = scratch.tile([C, N], f32, tag="scr", name="xs_sb")
                for h in range(2):
                    pxs = ps.tile([128, 512], f32, tag="mmA",
                                  name=f"pxs_{h}")
                    nc.tensor.matmul(pxs[:], cwT32[:],
                                     sxp[:, h * 512:(h + 1) * 512],
                                     start=True, stop=True)
                    nc.vector.tensor_scalar_add(
                        xs_sb[:, h * 512:(h + 1) * 512], pxs[:], Tcb[:])
                nc.sync.dma_start(xs_in[:], xs_sb[:])
                if sim_mode:
                    nc.sync.dma_start(xs_out[:], xs_in[:])
                else:
                    nc.gpsimd.collective_compute(
                        "AllReduce", OP.add,
                        replica_groups=[list(range(NCORES))],
                        ins=[xs_in.opt()], outs=[xs_out.opt()])
                nc.sync.dma_start(xs_full[:], xs_out[:])
                pid = nc.sync.partition_id()
                nc.sync.dma_start(xs_own[:], xs_out[:, bass.ts(pid, 128)])

                # own 128 adjacency rows
                r1 = scratch.tile([C, N], f32, tag="scr", name="r1")
                p1 = scratch.tile([C, N], f32, tag="scr", name="p1")
                p2 = scratch.tile([C, N], f32, tag="scr", name="p2")
                z = scratch.tile([C, N], f32, tag="scr", name="z")
                af = scratch.tile([C, N], f32, tag="scr", name="af")
                for src, pt_, st_ in ((mem32, p1, 0), (xs_full, p2, 1)):
                    for h in range(2):
                        pe = ps.tile([128, 512], f32, tag="mmA",
                                     name=f"pe_{st_}_{h}")
                        nc.tensor.matmul(pe[:], xs_own[:],
                                         src[:, h * 512:(h + 1) * 512],
                                         start=True, stop=True)
                        nc.scalar.activation(r1[:, h * 512:(h + 1) * 512],
                                             pe[:], AF.Relu, scale=SCALE)
                    mneg = colp.tile([C, 1], f32, tag=f"mneg{st_}")
                    nc.vector.tensor_reduce(mneg[:], r1[:], AX.X, OP.max,
                                            negate=True)
                    ssum = colp.tile([C, 1], f32, tag=f"ssum{st_}")
                    nc.scalar.activation(pt_[:], r1[:], AF.Exp,
                                         bias=mneg[:], accum_out=ssum[:])
                    rs = colp.tile([C, 1], f32, tag=f"rs{st_}")
                    nc.vector.reciprocal(rs[:], ssum[:])
                    wrs = colp.tile([C, 1], f32, tag=f"wrs{st_}")
                    nc.vector.tensor_tensor(wrs[:], rs[:],
                                            (w0b if st_ == 0 else w1b)[:],
                                            OP.mult)
                    if st_ == 0:
                        nc.vector.tensor_scalar_mul(z[:], pt_[:], wrs[:])
                    else:
                        nc.vector.scalar_tensor_tensor(z[:], pt_[:], wrs[:],
                                                       z[:], OP.mult, OP.add)
                nc.vector.tensor_scalar_add(z[:], z[:], fcbb[:])
                zmn = colp.tile([C, 1], f32, tag="zmn")
                nc.vector.tensor_reduce(zmn[:], z[:], AX.X, OP.max,
                                        negate=True)
                zs = colp.tile([C, 1], f32, tag="zs")
                pz = scratch.tile([C, N], f32, tag="scr", name="pz")
                nc.scalar.activation(pz[:], z[:], AF.Exp, bias=zmn[:],
                                     accum_out=zs[:])
                rzs = colp.tile([C, 1], f32, tag="rzs")
                nc.vector.reciprocal(rzs[:], zs[:])
                af = af
                nc.vector.tensor_scalar_mul(af[:], pz[:], rzs[:])
                # exact top-k mask
                mn = colp.tile([C, 1], f32, tag="mn")
                nc.vector.tensor_reduce(mn[:], af[:], AX.X, OP.min)
                isf = scratch.tile([C, N], f32, tag="scr", name="isf")
                nc.vector.tensor_scalar(isf[:], af[:], mn[:], None,
                                        OP.is_equal)
                nf = colp.tile([C, 1], f32, tag="nf")
                nc.vector.tensor_reduce(nf[:], isf[:], AX.X, OP.add)
                slots = colp.tile([C, 1], f32, tag="slots")
                nc.vector.tensor_scalar_add(slots[:], nf[:], float(-NK))
                pref = scratch.tile([C, N], f32, tag="scr", name="pref")
                nc.vector.tensor_tensor_scan(pref[:], isf[:], isf[:], 0.0,
                                             OP.add, OP.bypass)
                keep = scratch.tile([C, N], f32, tag="scr", name="keep")
                nc.vector.tensor_scalar(keep[:], pref[:], slots[:], None,
                                        OP.is_le)
                nc.vector.tensor_tensor(keep[:], keep[:], isf[:], OP.mult)
                gtm = scratch.tile([C, N], f32, tag="scr", name="gtm")
                nc.vector.tensor_scalar(gtm[:], af[:], mn[:], None, OP.is_gt)
                nc.vector.tensor_tensor(keep[:], keep[:], gtm[:], OP.add)
                adj_own = rnat.tile([C, N], f32r, tag="xr", name="adj_own")
                nc.vector.tensor_tensor(adj_own[:], af[:], keep[:], OP.mult)
                # AllGather full adjacency
                nc.sync.dma_start(ag_in[:], adj_own[:])
                if sim_mode:
                    for j in range(NT):
                        nc.sync.dma_start(
                            ag_out[j * 128:(j + 1) * 128, :], ag_in[:])
                else:
                    nc.gpsimd.collective_compute(
                        "AllGather", OP.bypass,
                        replica_groups=[list(range(NCORES))],
                        ins=[ag_in.opt()], outs=[ag_out.opt()])
                for j in range(NT):
                    nc.sync.dma_start(
                        adj_all[:, j * N:(j + 1) * N],
                        ag_out[j * 128:(j + 1) * 128, :])

            def pass2():
                for b in range(NBLK):
                    xcT = xcTp.tile([C, NT * 512], f32r, tag="xcT",
                                    name=f"xcT_p2_{b}")
                    for j in range(NT):
                        nc.sync.dma_start(
                            xcT[:, j * 512:(j + 1) * 512],
                            spill[j * 128:(j + 1) * 128,
                                  b * 512:(b + 1) * 512])
                    c1T = c1Tp.tile([C, NT * 512], f32r, tag="c1T",
                                    name=f"c1T_{b}")
                    # step 1: cur1T[j2] = sum_j adj[j,j2].T @ xcT[j]
                    for j2 in range(NT):
                        p1s = ps.tile([128, 512], f32, tag="mmA",
                                      name=f"p1s_{b}_{j2}")
                        for j in range(NT):
                            nc.tensor.matmul(
                                p1s[:],
                                adj_all[:, j * N + j2 * 128:
                                        j * N + (j2 + 1) * 128],
                                xcT[:, j * 512:(j + 1) * 512],
                                start=(j == 0), stop=(j == NT - 1))
                        nc.vector.tensor_copy(
                            c1T[:, j2 * 512:(j2 + 1) * 512], p1s[:])
                    # transpose cur1T -> cur1 natural (per t)
                    c1n = {}
                    for tl in range(TB):
                        c1n[tl] = rnat.tile([C, N], f32r, tag="xr",
                                            name=f"c1n_{b}_{tl}")
                        for g in range(2):
                            pt = ps.tile([128, 512], f32r, tag="tr",
                                         name=f"ptc_{b}_{tl}_{g}")
                            for jj in range(4):
                                j2 = g * 4 + jj
                                nc.tensor.transpose(
                                    pt[:, jj * 128:(jj + 1) * 128],
                                    c1T[:, j2 * 512 + tl * 128:
                                        j2 * 512 + (tl + 1) * 128],
                                    identr[:])
                            for jj in range(4):
                                j2 = g * 4 + jj
                                nc.vector.tensor_copy(
                                    c1n[tl][:, j2 * 128:(j2 + 1) * 128],
                                    pt[:, jj * 128:(jj + 1) * 128])
                    # step 2 + gcn + output (per t)
                    for tl in range(TB):
                        t = b * TB + tl
                        c2n = rnat2.tile([C, N], f32r, tag="xc",
                                         name=f"c2n_{b}_{tl}")
                        embt = embp.tile([C, N], f32, tag="embt",
                                         name=f"embt_{t}")
                        nc.sync.dma_start(embt[:], embi[:, t, :])
                        xsk = stream.tile([C, N], f32, tag="xt",
                                          name=f"xsk_{t}")
                        nc.sync.dma_start(xsk[:], xin[:, t, :])
                        for h in range(2):
                            p2s = ps.tile([128, 512], f32, tag="mmB",
                                          name=f"p2s_{b}_{tl}_{h}")
                            for j2 in range(NT):
                                nc.tensor.matmul(
                                    p2s[:],
                                    c1T[:, j2 * 512 + tl * 128:
                                        j2 * 512 + (tl + 1) * 128],
                                    adj_all[:, j2 * N + h * 512:
                                            j2 * N + (h + 1) * 512],
                                    start=(j2 == 0), stop=(j2 == NT - 1))
                            nc.vector.tensor_copy(
                                c2n[:, h * 512:(h + 1) * 512], p2s[:])
                        for h in range(2):
                            pg = ps.tile([128, 512], f32, tag="mmC",
                                         name=f"pg_{b}_{tl}_{h}")
                            nc.tensor.matmul(pg[:], gw1Tr[:],
                                             c1n[tl][:, h * 512:(h + 1) * 512],
                                             start=True, stop=False)
                            nc.tensor.matmul(pg[:], gw2Tr[:],
                                             c2n[:, h * 512:(h + 1) * 512],
                                             start=False, stop=True)
                            sl = slice(h * 512, (h + 1) * 512)
                            xg = scratch.tile([C, N], f32, tag="scr",
                                              name=f"xg_{b}_{tl}_{h}")
                            nc.vector.tensor_scalar_add(xg[:, sl], pg[:],
                                                        gb[:])
                            nc.vector.tensor_tensor(embt[:, sl], xg[:, sl],
                                                    embt[:, sl], OP.mult)
                            nc.vector.tensor_tensor(embt[:, sl], embt[:, sl],
                                                    xsk[:, sl], OP.add)
                        nc.sync.dma_start(outp[:, t, :], embt[:])

            for rep in range(R):
                if "p1" in parts:
                    pass1()
                if "adj" in parts:
                    adjacency()
                elif "p2" in parts:
                    # timing-only variant: fill adj_all from DRAM scratch
                    zf = scratch.tile([C, N], f32, tag="scr", name="zf")
                    nc.vector.memset(zf[:], 0.0009)
                    adj0 = rnat.tile([C, N], f32r, tag="xr", name="adj0")
                    nc.vector.tensor_copy(adj0[:], zf[:])
                    for j in range(NT):
                        nc.sync.dma_start(ag_out[j * 128:(j + 1) * 128, :],
                                          adj0[:])
                    for j in range(NT):
                        nc.sync.dma_start(adj_all[:, j * N:(j + 1) * N],
                                          ag_out[j * 128:(j + 1) * 128, :])
                if "p2" in parts:
                    pass2()
    nc.compile()
    return nc


def host_prep(x, conv_w, conv_b, memory, fc_w, fc_b, gcn_w, gcn_b, emb):
    """Build per-core in_maps from full inputs."""
    f = np.float32
    x = np.asarray(x, f)
    emb = np.asarray(emb, f)
    conv_w = np.asarray(conv_w, f)
    conv_b = np.asarray(conv_b, f)
    memory = np.asarray(memory, f)
    fc_w = np.asarray(fc_w, f)
    fc_b = np.asarray(fc_b, f)
    gcn_w = np.asarray(gcn_w, f)
    gcn_b = np.asarray(gcn_b, f)
    shared = {
        "memi": np.ascontiguousarray(memory),
        "cwTi": np.ascontiguousarray(conv_w.T),
        "gw1Ti": np.ascontiguousarray(gcn_w[:, :C].T),
        "gw2Ti": np.ascontiguousarray(gcn_w[:, C:].T),
        "identi": np.eye(C, dtype=f),
        "cbi": conv_b.reshape(C, 1).copy(),
        "Tcbi": (T * conv_b).reshape(C, 1).copy(),
        "gbi": gcn_b.reshape(C, 1).copy(),
        "w0bi": np.full((C, 1), fc_w[0], f),
        "w1bi": np.full((C, 1), fc_w[1], f),
        "fcbbi": np.full((C, 1), fc_b[0], f),
    }
    in_maps = []
    for c in range(NCORES):
        sl = slice(c * TS, (c + 1) * TS)
        m = dict(shared)
        m["xin"] = np.ascontiguousarray(x[:, :, sl].transpose(0, 2, 1))
        m["embi"] = np.ascontiguousarray(emb[:, :, sl].transpose(0, 2, 1))
        in_maps.append(m)
    return in_maps


_CACHE = {}


def kernel(**inputs) -> np.ndarray:
    if "nc" not in _CACHE:
        _CACHE["nc"] = build_kernel(R=1)
    nc = _CACHE["nc"]
    in_maps = host_prep(**inputs)
    res = bass_utils.run_bass_kernel_spmd(nc, in_maps,
                                          core_ids=list(range(NCORES)))
    out = np.empty((C, N, T), np.float32)
    for c in range(NCORES):
        out[:, :, c * TS:(c + 1) * TS] = \
            res.results[c]["outp"].transpose(0, 2, 1)
    return out

